# revision 3
# baseline (speedup 1.0000x reference)
"""Linformer attention Trainium2 kernel (optimized).

Sharding: 8 cores = 4 batches x 2 head-groups (8 heads each).
Reference reshapes (B,H,N,d)->(B,N,C) WITHOUT head transpose, so output row
r depends only on head h = r//256; each core produces an independent
final[b, 2048*g:(g+1)*2048, :] slice - no collectives.

v2 changes vs baseline:
  - DMA count ~729 -> ~70: host pre-tiles x and E into per-n-block
    contiguous layouts (1-2 MiB per DMA); single-DMA weight loads;
    bf16 tile-major output (host reassembles + casts).
  - Softmax rowsum broadcast via gpsimd.partition_broadcast in SBUF
    (was: DRAM round-trip per (head, j) holding PSUM slots hostage).
  - y matmuls packed in j-pairs (N=512); proj done per head-pair (N=512).
  - PSUM decoupling: py is copied to SBUF immediately (DVE), normalize
    chain runs out of SBUF; PE never waits on the normalize chain.
  - klm accumulation on GpSimd (idle), psum evacuations via nc.any.
  - dot matmuls of a head pair interleaved at partition bases 0/64
    (concurrent row-groups on HW).

Per-core layout:
  xTt   [8, 128, 8, 512]  bf16  [nb, p, ct, n']  x[b].T tiled
  wqkvT [1024, 1536]      bf16  (cols 0:512 q | 512:1024 k | 1024:1536 v)
  eTt   [8, 128, 32, 256] bf16  [nb, p, h*4+s, k]
  wprojT[1024, 1024]      bf16  Wproj.T
  bproj [1024, 1]         f32
  outTt [4, 8, 128, 512]  bf16  [g, co, p, (e q)]  (host reassembles)
"""

import sys

sys.path.insert(0, "/opt/trn_rl_repo")

import numpy as np
import ml_dtypes
from contextlib import ExitStack

import concourse.bass as bass
import concourse.tile as tile
from concourse import bacc
from concourse import mybir
from concourse.bass_utils import run_bass_kernel_spmd
from concourse.masks import make_identity

B, N, C = 4, 4096, 1024
H, K_LM = 16, 256
D = C // H  # 64
HPC = 8  # heads per core
F32 = mybir.dt.float32
BF16 = mybir.dt.bfloat16
FE = mybir.ActivationFunctionType

N_BLK = 512
N_BLKS = N // N_BLK  # 8


def build_program():
    nc = bacc.Bacc("TRN2", target_bir_lowering=False, debug=False, num_devices=8)

    xTt = nc.dram_tensor("xTt", [N_BLKS, 128, 8, N_BLK], BF16, kind="ExternalInput").ap()
    wqkvT = nc.dram_tensor("wqkvT", [C, 3 * D * HPC], BF16, kind="ExternalInput").ap()
    eTt = nc.dram_tensor("eTt", [N_BLKS, 128, 32, K_LM], BF16, kind="ExternalInput").ap()
    wprojT = nc.dram_tensor("wprojT", [C, C], BF16, kind="ExternalInput").ap()
    bproj = nc.dram_tensor("bproj", [C, 1], F32, kind="ExternalInput").ap()
    outTt = nc.dram_tensor("outTt", [4, 8, 128, 512], BF16, kind="ExternalOutput").ap()

    with tile.TileContext(nc) as tc, ExitStack() as ctx:
        singles = ctx.enter_context(tc.tile_pool(name="singles", bufs=1))
        qres = ctx.enter_context(tc.tile_pool(name="qres", bufs=1))

        ident = singles.tile([128, 128], BF16)
        make_identity(nc, ident)

        # bias_sb[p, co] = bproj[co*128 + p]
        bias_sb = singles.tile([128, 8], F32)
        nc.sync.dma_start(out=bias_sb, in_=bproj.rearrange("(a b) o -> b (a o)", b=128))

        # wproj resident for phase 2: wp_sb[p, ci, m] = wprojT[ci*128+p, m]
        # (DMA issued after phase-1 loads; only needed by phase 2)
        wp_sb = singles.tile([128, 8, C], BF16)

        qT = [qres.tile([128, N], BF16, tag=f"qT{i}", name=f"qT{i}") for i in range(4)]
        klm = qres.tile([128, HPC, K_LM], F32)  # rows 0:64 kT | 64:128 vT
        klmb = qres.tile([128, HPC, K_LM], BF16)
        klm_fix = qres.tile([128, 4, K_LM], BF16)  # odd heads, k/v halves swapped

        # ---------------- Phase 1: qkv + landmark projection ----------------
        with tc.tile_pool(name="wqp", bufs=1) as wqp, \
             tc.tile_pool(name="xtp", bufs=3) as xtp, \
             tc.tile_pool(name="etp", bufs=2) as etp, \
             tc.tile_pool(name="kvp", bufs=6) as kvp, \
             tc.tile_pool(name="ps1", bufs=2, space="PSUM") as ps1, \
             tc.tile_pool(name="ps_kv", bufs=3, space="PSUM") as ps_kv, \
             tc.tile_pool(name="ps_lm", bufs=2, space="PSUM") as ps_lm:

            w_sb = wqp.tile([128, 8, 3 * D * HPC], BF16)
            wq_view = wqkvT.rearrange("(a p) m -> p a m", p=128)

            for nb in range(N_BLKS):
                xt = xtp.tile([128, 8, N_BLK], BF16, tag="xt")
                if nb == 0:
                    # per-ct x/w chunks interleaved: the first q matmuls can
                    # start as soon as the first chunks land
                    for ct in range(8):
                        nc.sync.dma_start(out=xt[:, ct, :], in_=xTt[0, :, ct, :])
                        nc.sync.dma_start(out=w_sb[:, ct, :], in_=wq_view[:, ct, :])
                else:
                    nc.sync.dma_start(out=xt, in_=xTt[nb])
                et = etp.tile([128, 32, K_LM], BF16, tag="et")
                nc.sync.dma_start(out=et, in_=eTt[nb])

                # q: out[m(128), n(512)]
                for mt in range(4):
                    pq = ps1.tile([128, N_BLK], F32, tag="pq")
                    for ct in range(8):
                        nc.tensor.matmul(
                            pq,
                            w_sb[:, ct, mt * 128 : (mt + 1) * 128],
                            xt[:, ct, :],
                            start=(ct == 0),
                            stop=(ct == 7),
                        )
                    nc.any.tensor_copy(qT[mt][:, bass.ts(nb, N_BLK)], pq)

                # kv: out[n(128), m(1024)] ; kvt layout [p, h, half(k|v), d]
                kvs = []
                for s in range(4):
                    kvt = kvp.tile([128, HPC, 2, D], BF16, tag="kv")
                    for half in range(2):
                        pkv = ps_kv.tile([128, 512], F32, tag="pkv")
                        msl = bass.ds(512 + half * 512, 512)
                        for ct in range(8):
                            nc.tensor.matmul(
                                pkv,
                                xt[:, ct, s * 128 : (s + 1) * 128],
                                w_sb[:, ct, msl],
                                start=(ct == 0),
                                stop=(ct == 7),
                            )
                        nc.any.tensor_copy(
                            kvt[:, :, half, :], pkv.rearrange("p (h d) -> p h d", d=D)
                        )
                    kvs.append(kvt)

                # landmark accumulation per head
                for h in range(HPC):
                    plm = ps_lm.tile([128, K_LM], F32, tag="plm")
                    for s in range(4):
                        nc.tensor.matmul(
                            plm,
                            kvs[s][:, h, :, :],
                            et[:, h * 4 + s, :],
                            start=(s == 0),
                            stop=(s == 3),
                        )
                    if nb == 0:
                        nc.any.tensor_copy(klm[:, h, :], plm)
                    else:
                        nc.any.tensor_add(klm[:, h, :], klm[:, h, :], plm)

        # ---------------- Phase 2: attention + projection per head-pair -----
        with tc.tile_pool(name="expp", bufs=8) as expp, \
             tc.tile_pool(name="vop", bufs=16) as vop, \
             tc.tile_pool(name="yrp", bufs=3) as yrp, \
             tc.tile_pool(name="rsp", bufs=2) as rsp, \
             tc.tile_pool(name="rcp", bufs=2) as rcp, \
             tc.tile_pool(name="orp", bufs=3) as orp, \
             tc.tile_pool(name="tmpp", bufs=2) as tmpp, \
             tc.tile_pool(name="fop", bufs=4) as fop, \
             tc.tile_pool(name="ps_t", bufs=1, space="PSUM") as ps_t, \
             tc.tile_pool(name="ps_dot", bufs=2, space="PSUM") as ps_dot, \
             tc.tile_pool(name="ps_y", bufs=3, space="PSUM") as ps_y, \
             tc.tile_pool(name="ps_f", bufs=2, space="PSUM") as ps_f:

            nc.sync.dma_start(
                out=wp_sb, in_=wprojT.rearrange("(a p) m -> p a m", p=128)
            )

            # hoisted prep for ALL pairs: bf16 landmarks + odd-head k/v swap
            vones = {}
            for g in range(4):
                for e in range(2):
                    h = 2 * g + e
                    nc.any.tensor_copy(klmb[:, h, :], klm[:, h, :])
                    if e == 1:
                        nc.sync.dma_start(
                            out=klm_fix[64:128, g, :], in_=klmb[0:64, h, :]
                        )
                        nc.sync.dma_start(
                            out=klm_fix[0:64, g, :], in_=klmb[64:128, h, :]
                        )

            def gen_vones(g):
                # vones: [lm(128), 65] = [vlmT | 1] per (head, lm-half).
                # Generator so later pairs' transposes (PE) can interleave
                # into the ACT-paced dot stretch of earlier pairs.
                for e in range(2):
                    for half in range(2):
                        vt = vop.tile(
                            [128, 65], BF16, tag="vones", name=f"vt{g}_{e}_{half}"
                        )
                        pt = ps_t.tile([128, 64], BF16, tag="pt")
                        if e == 0:
                            vsrc = klmb[64:128, 2 * g, bass.ts(half, 128)]
                            idn = ident[64:128, 64:128]
                        else:
                            vsrc = klm_fix[0:64, g, bass.ts(half, 128)]
                            idn = ident[0:64, 0:64]
                        nc.tensor.transpose(pt, vsrc, idn)
                        nc.vector.tensor_copy(vt[:, 0:64], pt)
                        nc.vector.memset(vt[:, 64:65], 1.0)
                        vones[(g, e, half)] = vt
                        yield

            def gen_dot_exp(g, ex):
                # dot + exp, heads interleaved (row-groups 0:64 / 64:128).
                # Generator: yields after each MM+exp so the driver can
                # interleave proj matmuls of an earlier pair (the dot
                # stretch alone is ACT-paced at ~3x the PE time).
                for nt in range(8):
                    for half in range(2):
                        for e in range(2):
                            pd = ps_dot.tile([128, 512], F32, tag="pd")
                            if e == 0:
                                klmh = klmb[0:64, 2 * g, bass.ts(half, 128)]
                                qh = qT[g][0:64, bass.ts(nt, 512)]
                            else:
                                klmh = klm_fix[64:128, g, bass.ts(half, 128)]
                                qh = qT[g][64:128, bass.ts(nt, 512)]
                            nc.tensor.matmul(pd, klmh, qh, start=True, stop=True)
                            nc.scalar.activation(
                                ex[(e, half)][:, bass.ts(nt, 512)],
                                pd,
                                FE.Exp,
                                scale=0.125,
                            )
                            yield

            def gen_y(g, ex, ort):
                # y + rowsum (j-pairs); normalize fully in SBUF.
                # rowsum rows of a group of 2 j-pairs land in one yrh tile
                # (partition 64), get DMA-gathered to partition 0
                # (partition_broadcast only broadcasts absolute partition 0
                # on HW), broadcast on GpSimd, reciprocal in place, then one
                # wide mul per parity. Muls alternate DVE/GpSimd per group
                # to balance the two engines. Yields after each matmul.
                gsz = 2
                for e in range(2):
                    for ihg in range(4):
                        tmph = tmpp.tile(
                            [64, gsz, K_LM], BF16, tag="tmp",
                            name=f"tmp{g}_{e}_{ihg}",
                        )
                        yrh = yrp.tile([65, gsz, 2, K_LM], F32, tag="yrh")
                        for il in range(gsz):
                            i = ihg * gsz + il
                            py = ps_y.tile([65, 2, K_LM], F32, tag="py")
                            for half in range(2):
                                rhs = ex[(e, half)].rearrange(
                                    "p (q j) -> p j q", j=16
                                )[:, 2 * i : 2 * i + 2, :]
                                nc.tensor.matmul(
                                    py,
                                    vones[(g, e, half)],
                                    rhs,
                                    start=(half == 0),
                                    stop=(half == 1),
                                )
                                yield
                            nc.vector.tensor_copy(yrh[:, il, :, :], py)
                        rs0 = rsp.tile([1, gsz, 2, K_LM], F32, tag="rs0")
                        nc.sync.dma_start(out=rs0, in_=yrh[64:65, :, :, :])
                        rc = rcp.tile([64, gsz, 2, K_LM], F32, tag="rc")
                        nc.gpsimd.partition_broadcast(rc, rs0)
                        rc2 = rc.rearrange("p a b q -> p (a b q)")
                        nc.vector.reciprocal_approx_fast(out=rc2, in_=rc2)
                        isl = bass.ds(ihg * gsz, gsz)
                        eng_even = nc.vector if ihg % 2 == 0 else nc.gpsimd
                        eng_odd = nc.gpsimd if ihg % 2 == 0 else nc.vector
                        eng_even.tensor_mul(
                            ort[0:64, isl, e, :],
                            yrh[0:64, :, 0, :],
                            rc[:, :, 0, :],
                        )
                        eng_odd.tensor_mul(
                            tmph[:, :, :],
                            yrh[0:64, :, 1, :],
                            rc[:, :, 1, :],
                        )
                        nc.sync.dma_start(
                            out=ort[64:128, isl, e, :], in_=tmph
                        )

            def gen_proj(g, ort):
                # projection for this head pair (512 output rows);
                # yields after each MM for interleaving
                for co in range(8):
                    pf = ps_f.tile([128, 512], F32, tag="pf")
                    for ci in range(8):
                        nc.tensor.matmul(
                            pf,
                            wp_sb[:, ci, co * 128 : (co + 1) * 128],
                            ort[:, ci, :, :],
                            start=(ci == 0),
                            stop=(ci == 7),
                        )
                        yield
                    fo = fop.tile([128, 512], BF16, tag="fo")
                    nc.scalar.activation(
                        fo, pf, FE.Identity, bias=bias_sb[:, co : co + 1]
                    )
                    nc.sync.dma_start(out=outTt[g, co], in_=fo)

            # 3-deep software pipeline: in driver round g, the in-order PE
            # stream interleaves dot(g) (ACT-paced), y(g-1) (DVE-paced,
            # front-loaded: its ex is complete), and proj(g-2) (back-loaded:
            # its ort finishes during this round).  vones(g+1) transposes
            # ride along.  Steady-state rounds carry ~27us of PE work.
            orts = {}
            exs = {}
            for _ in gen_vones(0):
                pass
            for g in range(6):
                gens = []
                if g < 4:
                    exs[g] = {
                        (e, half): expp.tile(
                            [128, N], BF16, tag="ex", name=f"ex{g}_{e}_{half}"
                        )
                        for e in range(2)
                        for half in range(2)
                    }
                    gens.append((gen_dot_exp(g, exs[g]), 1.0))
                if 0 <= g - 1 < 4:
                    orts[g - 1] = orp.tile(
                        [128, 8, 2, K_LM], BF16, tag="ort", name=f"ort{g - 1}"
                    )
                    gens.append((gen_y(g - 1, exs.pop(g - 1), orts[g - 1]), 1.2))
                if 0 <= g - 2 < 4:
                    gens.append((gen_proj(g - 2, orts.pop(g - 2)), 0.8))
                if g < 3:
                    gens.append((gen_vones(g + 1), 3.0))
                # weighted round-robin: rate = yields per iteration;
                # higher rate drains a generator earlier in the round
                credits = [0.0 for _ in gens]
                done = [False] * len(gens)
                while not all(done):
                    for idx, (gen, rate) in enumerate(gens):
                        if done[idx]:
                            continue
                        credits[idx] += rate
                        while credits[idx] >= 1.0:
                            credits[idx] -= 1.0
                            if next(gen, StopIteration) is StopIteration:
                                done[idx] = True
                                break

    nc.compile()
    return nc


_NC_CACHE = None


def make_in_maps(x, Wqkv, E, Wproj, bproj_v):
    wprojT = np.ascontiguousarray(Wproj.T).astype(ml_dtypes.bfloat16)
    bp = np.ascontiguousarray(bproj_v.reshape(C, 1)).astype(np.float32)

    in_maps = []
    for cid in range(8):
        b, g = cid // 2, cid % 2
        rows = np.concatenate(
            [
                np.arange(s * C + g * HPC * D, s * C + g * HPC * D + HPC * D)
                for s in range(3)
            ]
        )
        # xTt[nb, p, ct, n'] = x[b, nb*512+n', ct*128+p]
        xtt = np.ascontiguousarray(
            x[b].reshape(8, 512, 8, 128).transpose(0, 3, 2, 1)
        ).astype(ml_dtypes.bfloat16)
        # eTt[nb, p, h*4+s, k] = E[g*8+h, k, nb*512+s*128+p]
        eg = E[g * HPC : (g + 1) * HPC]  # [8, 256, 4096]
        ett = np.ascontiguousarray(
            eg.transpose(2, 0, 1)  # [n, h, k]
            .reshape(8, 4, 128, HPC, K_LM)  # [nb, s, p, h, k]
            .transpose(0, 2, 3, 1, 4)  # [nb, p, h, s, k]
            .reshape(8, 128, 32, K_LM)
        ).astype(ml_dtypes.bfloat16)
        in_maps.append(
            {
                "xTt": xtt,
                "wqkvT": np.ascontiguousarray(Wqkv[rows].T).astype(ml_dtypes.bfloat16),
                "eTt": ett,
                "wprojT": wprojT,
                "bproj": bp,
            }
        )
    return in_maps


def kernel(x, Wqkv, E, Wproj, bproj, **_):
    global _NC_CACHE
    x = np.asarray(x, dtype=np.float32)
    Wqkv = np.asarray(Wqkv, dtype=np.float32)
    E = np.asarray(E, dtype=np.float32)
    Wproj = np.asarray(Wproj, dtype=np.float32)
    bproj = np.asarray(bproj, dtype=np.float32)

    if _NC_CACHE is None:
        _NC_CACHE = build_program()
    nc = _NC_CACHE

    in_maps = make_in_maps(x, Wqkv, E, Wproj, bproj)
    res = run_bass_kernel_spmd(nc, in_maps, core_ids=list(range(8)))

    out = np.empty((B, N, C), dtype=np.float32)
    for cid in range(8):
        b, g = cid // 2, cid % 2
        arr = np.asarray(res.results[cid]["outTt"], dtype=np.float32)
        # [gp, co, p, e, q] -> rows (2gp+e)*256+q, cols co*128+p
        out[b, g * 2048 : (g + 1) * 2048, :] = (
            arr.reshape(4, 8, 128, 2, 256)
            .transpose(0, 3, 4, 1, 2)
            .reshape(2048, 1024)
        )
    return out


# revision 4
# speedup vs baseline: 1.0040x; 1.0040x over previous
"""Linformer attention Trainium2 kernel (optimized).

Sharding: 8 cores = 4 batches x 2 head-groups (8 heads each).
Reference reshapes (B,H,N,d)->(B,N,C) WITHOUT head transpose, so output row
r depends only on head h = r//256; each core produces an independent
final[b, 2048*g:(g+1)*2048, :] slice - no collectives.

v2 changes vs baseline:
  - DMA count ~729 -> ~70: host pre-tiles x and E into per-n-block
    contiguous layouts (1-2 MiB per DMA); single-DMA weight loads;
    bf16 tile-major output (host reassembles + casts).
  - Softmax rowsum broadcast via gpsimd.partition_broadcast in SBUF
    (was: DRAM round-trip per (head, j) holding PSUM slots hostage).
  - y matmuls packed in j-pairs (N=512); proj done per head-pair (N=512).
  - PSUM decoupling: py is copied to SBUF immediately (DVE), normalize
    chain runs out of SBUF; PE never waits on the normalize chain.
  - klm accumulation on GpSimd (idle), psum evacuations via nc.any.
  - dot matmuls of a head pair interleaved at partition bases 0/64
    (concurrent row-groups on HW).

Per-core layout:
  xTt   [8, 128, 8, 512]  bf16  [nb, p, ct, n']  x[b].T tiled
  wqkvT [1024, 1536]      bf16  (cols 0:512 q | 512:1024 k | 1024:1536 v)
  eTt   [8, 128, 32, 256] bf16  [nb, p, h*4+s, k]
  wprojT[1024, 1024]      bf16  Wproj.T
  bproj [1024, 1]         f32
  outTt [4, 8, 128, 512]  bf16  [g, co, p, (e q)]  (host reassembles)
"""

import sys

sys.path.insert(0, "/opt/trn_rl_repo")

import numpy as np
import ml_dtypes
from contextlib import ExitStack

import concourse.bass as bass
import concourse.tile as tile
from concourse import bacc
from concourse import mybir
from concourse.bass_utils import run_bass_kernel_spmd
from concourse.masks import make_identity

B, N, C = 4, 4096, 1024
H, K_LM = 16, 256
D = C // H  # 64
HPC = 8  # heads per core
F32 = mybir.dt.float32
BF16 = mybir.dt.bfloat16
FE = mybir.ActivationFunctionType

N_BLK = 512
N_BLKS = N // N_BLK  # 8


def build_program():
    nc = bacc.Bacc("TRN2", target_bir_lowering=False, debug=False, num_devices=8)

    xTt = nc.dram_tensor("xTt", [N_BLKS, 128, 8, N_BLK], BF16, kind="ExternalInput").ap()
    wqkvT = nc.dram_tensor("wqkvT", [C, 3 * D * HPC], BF16, kind="ExternalInput").ap()
    eTt = nc.dram_tensor("eTt", [N_BLKS, 128, 32, K_LM], BF16, kind="ExternalInput").ap()
    wprojT = nc.dram_tensor("wprojT", [C, C], BF16, kind="ExternalInput").ap()
    bproj = nc.dram_tensor("bproj", [C, 1], F32, kind="ExternalInput").ap()
    outTt = nc.dram_tensor("outTt", [4, 8, 128, 512], BF16, kind="ExternalOutput").ap()

    with tile.TileContext(nc) as tc, ExitStack() as ctx:
        singles = ctx.enter_context(tc.tile_pool(name="singles", bufs=1))
        qres = ctx.enter_context(tc.tile_pool(name="qres", bufs=1))

        ident = singles.tile([128, 128], BF16)
        make_identity(nc, ident)

        # bias_sb[p, co] = bproj[co*128 + p]
        bias_sb = singles.tile([128, 8], F32)
        nc.sync.dma_start(out=bias_sb, in_=bproj.rearrange("(a b) o -> b (a o)", b=128))

        # wproj resident for phase 2: wp_sb[p, ci, m] = wprojT[ci*128+p, m]
        # (DMA issued after phase-1 loads; only needed by phase 2)
        wp_sb = singles.tile([128, 8, C], BF16)

        qT = [qres.tile([128, N], BF16, tag=f"qT{i}", name=f"qT{i}") for i in range(4)]
        klm = qres.tile([128, HPC, K_LM], F32)  # rows 0:64 kT | 64:128 vT
        klmb = qres.tile([128, HPC, K_LM], BF16)
        klm_fix = qres.tile([128, 4, K_LM], BF16)  # odd heads, k/v halves swapped

        # ---------------- Phase 1: qkv + landmark projection ----------------
        with tc.tile_pool(name="wqp", bufs=1) as wqp, \
             tc.tile_pool(name="xtp", bufs=4) as xtp, \
             tc.tile_pool(name="etp", bufs=3) as etp, \
             tc.tile_pool(name="kvp", bufs=8) as kvp, \
             tc.tile_pool(name="ps1", bufs=2, space="PSUM") as ps1, \
             tc.tile_pool(name="ps_kv", bufs=3, space="PSUM") as ps_kv, \
             tc.tile_pool(name="ps_lm", bufs=3, space="PSUM") as ps_lm:

            w_sb = wqp.tile([128, 8, 3 * D * HPC], BF16)
            wq_view = wqkvT.rearrange("(a p) m -> p a m", p=128)

            for nb in range(N_BLKS):
                xt = xtp.tile([128, 8, N_BLK], BF16, tag="xt")
                if nb == 0:
                    # per-ct x/w chunks interleaved: the first q matmuls can
                    # start as soon as the first chunks land
                    for ct in range(8):
                        nc.sync.dma_start(out=xt[:, ct, :], in_=xTt[0, :, ct, :])
                        nc.sync.dma_start(out=w_sb[:, ct, :], in_=wq_view[:, ct, :])
                else:
                    nc.sync.dma_start(out=xt, in_=xTt[nb])
                et = etp.tile([128, 32, K_LM], BF16, tag="et")
                nc.sync.dma_start(out=et, in_=eTt[nb])

                # q: out[m(128), n(512)]
                for mt in range(4):
                    pq = ps1.tile([128, N_BLK], F32, tag="pq")
                    for ct in range(8):
                        nc.tensor.matmul(
                            pq,
                            w_sb[:, ct, mt * 128 : (mt + 1) * 128],
                            xt[:, ct, :],
                            start=(ct == 0),
                            stop=(ct == 7),
                        )
                    nc.any.tensor_copy(qT[mt][:, bass.ts(nb, N_BLK)], pq)

                # kv: out[n(128), m(1024)] ; kvt layout [p, h, half(k|v), d]
                kvs = []
                for s in range(4):
                    kvt = kvp.tile([128, HPC, 2, D], BF16, tag="kv")
                    for half in range(2):
                        pkv = ps_kv.tile([128, 512], F32, tag="pkv")
                        msl = bass.ds(512 + half * 512, 512)
                        for ct in range(8):
                            nc.tensor.matmul(
                                pkv,
                                xt[:, ct, s * 128 : (s + 1) * 128],
                                w_sb[:, ct, msl],
                                start=(ct == 0),
                                stop=(ct == 7),
                            )
                        nc.any.tensor_copy(
                            kvt[:, :, half, :], pkv.rearrange("p (h d) -> p h d", d=D)
                        )
                    kvs.append(kvt)

                # landmark accumulation per head
                for h in range(HPC):
                    plm = ps_lm.tile([128, K_LM], F32, tag="plm")
                    for s in range(4):
                        nc.tensor.matmul(
                            plm,
                            kvs[s][:, h, :, :],
                            et[:, h * 4 + s, :],
                            start=(s == 0),
                            stop=(s == 3),
                        )
                    if nb == 0:
                        nc.any.tensor_copy(klm[:, h, :], plm)
                    else:
                        nc.any.tensor_add(klm[:, h, :], klm[:, h, :], plm)

        # ---------------- Phase 2: attention + projection per head-pair -----
        with tc.tile_pool(name="expp", bufs=8) as expp, \
             tc.tile_pool(name="vop", bufs=16) as vop, \
             tc.tile_pool(name="yrp", bufs=3) as yrp, \
             tc.tile_pool(name="rsp", bufs=2) as rsp, \
             tc.tile_pool(name="rcp", bufs=2) as rcp, \
             tc.tile_pool(name="orp", bufs=3) as orp, \
             tc.tile_pool(name="tmpp", bufs=2) as tmpp, \
             tc.tile_pool(name="fop", bufs=4) as fop, \
             tc.tile_pool(name="ps_t", bufs=1, space="PSUM") as ps_t, \
             tc.tile_pool(name="ps_dot", bufs=2, space="PSUM") as ps_dot, \
             tc.tile_pool(name="ps_y", bufs=3, space="PSUM") as ps_y, \
             tc.tile_pool(name="ps_f", bufs=2, space="PSUM") as ps_f:

            nc.sync.dma_start(
                out=wp_sb, in_=wprojT.rearrange("(a p) m -> p a m", p=128)
            )

            # hoisted prep for ALL pairs: bf16 landmarks + odd-head k/v swap
            vones = {}
            for g in range(4):
                for e in range(2):
                    h = 2 * g + e
                    nc.any.tensor_copy(klmb[:, h, :], klm[:, h, :])
                    if e == 1:
                        nc.sync.dma_start(
                            out=klm_fix[64:128, g, :], in_=klmb[0:64, h, :]
                        )
                        nc.sync.dma_start(
                            out=klm_fix[0:64, g, :], in_=klmb[64:128, h, :]
                        )

            def gen_vones(g):
                # vones: [lm(128), 65] = [vlmT | 1] per (head, lm-half).
                # Generator so later pairs' transposes (PE) can interleave
                # into the ACT-paced dot stretch of earlier pairs.
                for e in range(2):
                    for half in range(2):
                        vt = vop.tile(
                            [128, 65], BF16, tag="vones", name=f"vt{g}_{e}_{half}"
                        )
                        pt = ps_t.tile([128, 64], BF16, tag="pt")
                        if e == 0:
                            vsrc = klmb[64:128, 2 * g, bass.ts(half, 128)]
                            idn = ident[64:128, 64:128]
                        else:
                            vsrc = klm_fix[0:64, g, bass.ts(half, 128)]
                            idn = ident[0:64, 0:64]
                        nc.tensor.transpose(pt, vsrc, idn)
                        nc.vector.tensor_copy(vt[:, 0:64], pt)
                        nc.vector.memset(vt[:, 64:65], 1.0)
                        vones[(g, e, half)] = vt
                        yield

            def gen_dot_exp(g, ex):
                # dot + exp, heads interleaved (row-groups 0:64 / 64:128).
                # Generator: yields after each MM+exp so the driver can
                # interleave proj matmuls of an earlier pair (the dot
                # stretch alone is ACT-paced at ~3x the PE time).
                for nt in range(8):
                    for half in range(2):
                        for e in range(2):
                            pd = ps_dot.tile([128, 512], F32, tag="pd")
                            if e == 0:
                                klmh = klmb[0:64, 2 * g, bass.ts(half, 128)]
                                qh = qT[g][0:64, bass.ts(nt, 512)]
                            else:
                                klmh = klm_fix[64:128, g, bass.ts(half, 128)]
                                qh = qT[g][64:128, bass.ts(nt, 512)]
                            nc.tensor.matmul(pd, klmh, qh, start=True, stop=True)
                            nc.scalar.activation(
                                ex[(e, half)][:, bass.ts(nt, 512)],
                                pd,
                                FE.Exp,
                                scale=0.125,
                            )
                            yield

            def gen_y(g, ex, ort):
                # y + rowsum (j-pairs); normalize fully in SBUF.
                # rowsum rows of a group of 2 j-pairs land in one yrh tile
                # (partition 64), get DMA-gathered to partition 0
                # (partition_broadcast only broadcasts absolute partition 0
                # on HW), broadcast on GpSimd, reciprocal in place, then one
                # wide mul per parity. Muls alternate DVE/GpSimd per group
                # to balance the two engines. Yields after each matmul.
                gsz = 2
                for e in range(2):
                    for ihg in range(4):
                        tmph = tmpp.tile(
                            [64, gsz, K_LM], BF16, tag="tmp",
                            name=f"tmp{g}_{e}_{ihg}",
                        )
                        yrh = yrp.tile([65, gsz, 2, K_LM], F32, tag="yrh")
                        for il in range(gsz):
                            i = ihg * gsz + il
                            py = ps_y.tile([65, 2, K_LM], F32, tag="py")
                            for half in range(2):
                                rhs = ex[(e, half)].rearrange(
                                    "p (q j) -> p j q", j=16
                                )[:, 2 * i : 2 * i + 2, :]
                                nc.tensor.matmul(
                                    py,
                                    vones[(g, e, half)],
                                    rhs,
                                    start=(half == 0),
                                    stop=(half == 1),
                                )
                                yield
                            nc.vector.tensor_copy(yrh[:, il, :, :], py)
                        rs0 = rsp.tile([1, gsz, 2, K_LM], F32, tag="rs0")
                        nc.sync.dma_start(out=rs0, in_=yrh[64:65, :, :, :])
                        rc = rcp.tile([64, gsz, 2, K_LM], F32, tag="rc")
                        nc.gpsimd.partition_broadcast(rc, rs0)
                        rc2 = rc.rearrange("p a b q -> p (a b q)")
                        nc.vector.reciprocal_approx_fast(out=rc2, in_=rc2)
                        isl = bass.ds(ihg * gsz, gsz)
                        eng_even = nc.vector if ihg % 2 == 0 else nc.gpsimd
                        eng_odd = nc.gpsimd if ihg % 2 == 0 else nc.vector
                        eng_even.tensor_mul(
                            ort[0:64, isl, e, :],
                            yrh[0:64, :, 0, :],
                            rc[:, :, 0, :],
                        )
                        eng_odd.tensor_mul(
                            tmph[:, :, :],
                            yrh[0:64, :, 1, :],
                            rc[:, :, 1, :],
                        )
                        nc.sync.dma_start(
                            out=ort[64:128, isl, e, :], in_=tmph
                        )

            def gen_proj(g, ort):
                # projection for this head pair (512 output rows);
                # yields after each MM for interleaving
                for co in range(8):
                    pf = ps_f.tile([128, 512], F32, tag="pf")
                    for ci in range(8):
                        nc.tensor.matmul(
                            pf,
                            wp_sb[:, ci, co * 128 : (co + 1) * 128],
                            ort[:, ci, :, :],
                            start=(ci == 0),
                            stop=(ci == 7),
                        )
                        yield
                    fo = fop.tile([128, 512], BF16, tag="fo")
                    nc.scalar.activation(
                        fo, pf, FE.Identity, bias=bias_sb[:, co : co + 1]
                    )
                    nc.sync.dma_start(out=outTt[g, co], in_=fo)

            # 3-deep software pipeline: in driver round g, the in-order PE
            # stream interleaves dot(g) (ACT-paced), y(g-1) (DVE-paced,
            # front-loaded: its ex is complete), and proj(g-2) (back-loaded:
            # its ort finishes during this round).  vones(g+1) transposes
            # ride along.  Steady-state rounds carry ~27us of PE work.
            orts = {}
            exs = {}
            for _ in gen_vones(0):
                pass
            for g in range(6):
                gens = []
                if g < 4:
                    exs[g] = {
                        (e, half): expp.tile(
                            [128, N], BF16, tag="ex", name=f"ex{g}_{e}_{half}"
                        )
                        for e in range(2)
                        for half in range(2)
                    }
                    gens.append((gen_dot_exp(g, exs[g]), 1.0))
                if 0 <= g - 1 < 4:
                    orts[g - 1] = orp.tile(
                        [128, 8, 2, K_LM], BF16, tag="ort", name=f"ort{g - 1}"
                    )
                    gens.append((gen_y(g - 1, exs.pop(g - 1), orts[g - 1]), 1.2))
                if 0 <= g - 2 < 4:
                    gens.append((gen_proj(g - 2, orts.pop(g - 2)), 0.8))
                if g < 3:
                    gens.append((gen_vones(g + 1), 3.0))
                # weighted round-robin: rate = yields per iteration;
                # higher rate drains a generator earlier in the round
                credits = [0.0 for _ in gens]
                done = [False] * len(gens)
                while not all(done):
                    for idx, (gen, rate) in enumerate(gens):
                        if done[idx]:
                            continue
                        credits[idx] += rate
                        while credits[idx] >= 1.0:
                            credits[idx] -= 1.0
                            if next(gen, StopIteration) is StopIteration:
                                done[idx] = True
                                break

    nc.compile()
    return nc


_NC_CACHE = None


def make_in_maps(x, Wqkv, E, Wproj, bproj_v):
    wprojT = np.ascontiguousarray(Wproj.T).astype(ml_dtypes.bfloat16)
    bp = np.ascontiguousarray(bproj_v.reshape(C, 1)).astype(np.float32)

    in_maps = []
    for cid in range(8):
        b, g = cid // 2, cid % 2
        rows = np.concatenate(
            [
                np.arange(s * C + g * HPC * D, s * C + g * HPC * D + HPC * D)
                for s in range(3)
            ]
        )
        # xTt[nb, p, ct, n'] = x[b, nb*512+n', ct*128+p]
        xtt = np.ascontiguousarray(
            x[b].reshape(8, 512, 8, 128).transpose(0, 3, 2, 1)
        ).astype(ml_dtypes.bfloat16)
        # eTt[nb, p, h*4+s, k] = E[g*8+h, k, nb*512+s*128+p]
        eg = E[g * HPC : (g + 1) * HPC]  # [8, 256, 4096]
        ett = np.ascontiguousarray(
            eg.transpose(2, 0, 1)  # [n, h, k]
            .reshape(8, 4, 128, HPC, K_LM)  # [nb, s, p, h, k]
            .transpose(0, 2, 3, 1, 4)  # [nb, p, h, s, k]
            .reshape(8, 128, 32, K_LM)
        ).astype(ml_dtypes.bfloat16)
        in_maps.append(
            {
                "xTt": xtt,
                "wqkvT": np.ascontiguousarray(Wqkv[rows].T).astype(ml_dtypes.bfloat16),
                "eTt": ett,
                "wprojT": wprojT,
                "bproj": bp,
            }
        )
    return in_maps


def kernel(x, Wqkv, E, Wproj, bproj, **_):
    global _NC_CACHE
    x = np.asarray(x, dtype=np.float32)
    Wqkv = np.asarray(Wqkv, dtype=np.float32)
    E = np.asarray(E, dtype=np.float32)
    Wproj = np.asarray(Wproj, dtype=np.float32)
    bproj = np.asarray(bproj, dtype=np.float32)

    if _NC_CACHE is None:
        _NC_CACHE = build_program()
    nc = _NC_CACHE

    in_maps = make_in_maps(x, Wqkv, E, Wproj, bproj)
    res = run_bass_kernel_spmd(nc, in_maps, core_ids=list(range(8)))

    out = np.empty((B, N, C), dtype=np.float32)
    for cid in range(8):
        b, g = cid // 2, cid % 2
        arr = np.asarray(res.results[cid]["outTt"], dtype=np.float32)
        # [gp, co, p, e, q] -> rows (2gp+e)*256+q, cols co*128+p
        out[b, g * 2048 : (g + 1) * 2048, :] = (
            arr.reshape(4, 8, 128, 2, 256)
            .transpose(0, 3, 4, 1, 2)
            .reshape(2048, 1024)
        )
    return out


# revision 5
# speedup vs baseline: 1.0132x; 1.0092x over previous
"""Linformer attention Trainium2 kernel (optimized).

Sharding: 8 cores = 4 batches x 2 head-groups (8 heads each).
Reference reshapes (B,H,N,d)->(B,N,C) WITHOUT head transpose, so output row
r depends only on head h = r//256; each core produces an independent
final[b, 2048*g:(g+1)*2048, :] slice - no collectives.

v2 changes vs baseline:
  - DMA count ~729 -> ~70: host pre-tiles x and E into per-n-block
    contiguous layouts (1-2 MiB per DMA); single-DMA weight loads;
    bf16 tile-major output (host reassembles + casts).
  - Softmax rowsum broadcast via gpsimd.partition_broadcast in SBUF
    (was: DRAM round-trip per (head, j) holding PSUM slots hostage).
  - y matmuls packed in j-pairs (N=512); proj done per head-pair (N=512).
  - PSUM decoupling: py is copied to SBUF immediately (DVE), normalize
    chain runs out of SBUF; PE never waits on the normalize chain.
  - klm accumulation on GpSimd (idle), psum evacuations via nc.any.
  - dot matmuls of a head pair interleaved at partition bases 0/64
    (concurrent row-groups on HW).

Per-core layout:
  xTt   [8, 128, 8, 512]  bf16  [nb, p, ct, n']  x[b].T tiled
  wqkvT [1024, 1536]      bf16  (cols 0:512 q | 512:1024 k | 1024:1536 v)
  eTt   [8, 128, 32, 256] bf16  [nb, p, h*4+s, k]
  wprojT[1024, 1024]      bf16  Wproj.T
  bproj [1024, 1]         f32
  outTt [4, 8, 128, 512]  bf16  [g, co, p, (e q)]  (host reassembles)
"""

import sys

sys.path.insert(0, "/opt/trn_rl_repo")

import numpy as np
import ml_dtypes
from contextlib import ExitStack

import concourse.bass as bass
import concourse.tile as tile
from concourse import bacc
from concourse import mybir
from concourse.bass_utils import run_bass_kernel_spmd
from concourse.masks import make_identity

B, N, C = 4, 4096, 1024
H, K_LM = 16, 256
D = C // H  # 64
HPC = 8  # heads per core
F32 = mybir.dt.float32
BF16 = mybir.dt.bfloat16
FE = mybir.ActivationFunctionType

N_BLK = 512
N_BLKS = N // N_BLK  # 8


def build_program():
    nc = bacc.Bacc("TRN2", target_bir_lowering=False, debug=False, num_devices=8)

    xTt = nc.dram_tensor("xTt", [N_BLKS, 128, 8, N_BLK], BF16, kind="ExternalInput").ap()
    wqkvT = nc.dram_tensor("wqkvT", [C, 3 * D * HPC], BF16, kind="ExternalInput").ap()
    eTt = nc.dram_tensor("eTt", [N_BLKS, 128, 32, K_LM], BF16, kind="ExternalInput").ap()
    wprojT = nc.dram_tensor("wprojT", [C, C], BF16, kind="ExternalInput").ap()
    bproj = nc.dram_tensor("bproj", [C, 1], F32, kind="ExternalInput").ap()
    outTt = nc.dram_tensor("outTt", [4, 8, 128, 512], BF16, kind="ExternalOutput").ap()

    with tile.TileContext(nc) as tc, ExitStack() as ctx:
        singles = ctx.enter_context(tc.tile_pool(name="singles", bufs=1))
        qres = ctx.enter_context(tc.tile_pool(name="qres", bufs=1))

        ident = singles.tile([128, 128], BF16)
        make_identity(nc, ident)

        # bias_sb[p, co] = bproj[co*128 + p]  (DMA deferred to phase 2)
        bias_sb = singles.tile([128, 8], F32)

        # wproj resident for phase 2: wp_sb[p, ci, m] = wprojT[ci*128+p, m]
        # (DMA issued after phase-1 loads; only needed by phase 2)
        wp_sb = singles.tile([128, 8, C], BF16)

        qT = [qres.tile([128, N], BF16, tag=f"qT{i}", name=f"qT{i}") for i in range(4)]
        klm = qres.tile([128, HPC, K_LM], F32)  # rows 0:64 kT | 64:128 vT
        klmb = qres.tile([128, HPC, K_LM], BF16)
        klm_fix = qres.tile([128, 4, K_LM], BF16)  # odd heads, k/v halves swapped

        # ---------------- Phase 1: qkv + landmark projection ----------------
        with tc.tile_pool(name="wqp", bufs=1) as wqp, \
             tc.tile_pool(name="xtp", bufs=4) as xtp, \
             tc.tile_pool(name="etp", bufs=3) as etp, \
             tc.tile_pool(name="kvp", bufs=8) as kvp, \
             tc.tile_pool(name="ps1", bufs=2, space="PSUM") as ps1, \
             tc.tile_pool(name="ps_kv", bufs=2, space="PSUM") as ps_kv, \
             tc.tile_pool(name="ps_lm", bufs=4, space="PSUM") as ps_lm:

            w_sb = wqp.tile([128, 8, 3 * D * HPC], BF16)
            wq_view = wqkvT.rearrange("(a p) m -> p a m", p=128)

            for nb in range(N_BLKS):
                xt = xtp.tile([128, 8, N_BLK], BF16, tag="xt")
                if nb == 0:
                    # per-ct x/w chunks interleaved: the first q matmuls can
                    # start as soon as the first chunks land
                    for ct in range(8):
                        nc.sync.dma_start(out=xt[:, ct, :], in_=xTt[0, :, ct, :])
                        nc.sync.dma_start(out=w_sb[:, ct, :], in_=wq_view[:, ct, :])
                else:
                    nc.sync.dma_start(out=xt, in_=xTt[nb])
                et = etp.tile([128, 32, K_LM], BF16, tag="et")
                nc.sync.dma_start(out=et, in_=eTt[nb])

                # q: out[m(128), n(512)]
                for mt in range(4):
                    pq = ps1.tile([128, N_BLK], F32, tag="pq")
                    for ct in range(8):
                        nc.tensor.matmul(
                            pq,
                            w_sb[:, ct, mt * 128 : (mt + 1) * 128],
                            xt[:, ct, :],
                            start=(ct == 0),
                            stop=(ct == 7),
                        )
                    nc.any.tensor_copy(qT[mt][:, bass.ts(nb, N_BLK)], pq)

                # kv: out[n(128), m(1024)] ; kvt layout [p, h, half(k|v), d]
                kvs = []
                for s in range(4):
                    kvt = kvp.tile([128, HPC, 2, D], BF16, tag="kv")
                    for half in range(2):
                        pkv = ps_kv.tile([128, 512], F32, tag="pkv")
                        msl = bass.ds(512 + half * 512, 512)
                        for ct in range(8):
                            nc.tensor.matmul(
                                pkv,
                                xt[:, ct, s * 128 : (s + 1) * 128],
                                w_sb[:, ct, msl],
                                start=(ct == 0),
                                stop=(ct == 7),
                            )
                        nc.any.tensor_copy(
                            kvt[:, :, half, :], pkv.rearrange("p (h d) -> p h d", d=D)
                        )
                    kvs.append(kvt)

                # landmark accumulation per head
                for h in range(HPC):
                    plm = ps_lm.tile([128, K_LM], F32, tag="plm")
                    for s in range(4):
                        nc.tensor.matmul(
                            plm,
                            kvs[s][:, h, :, :],
                            et[:, h * 4 + s, :],
                            start=(s == 0),
                            stop=(s == 3),
                        )
                    if nb == 0:
                        nc.any.tensor_copy(klm[:, h, :], plm)
                    else:
                        nc.any.tensor_add(klm[:, h, :], klm[:, h, :], plm)

        # ---------------- Phase 2: attention + projection per head-pair -----
        with tc.tile_pool(name="expp", bufs=8) as expp, \
             tc.tile_pool(name="vop", bufs=16) as vop, \
             tc.tile_pool(name="yrp", bufs=4) as yrp, \
             tc.tile_pool(name="rsp", bufs=2) as rsp, \
             tc.tile_pool(name="rcp", bufs=2) as rcp, \
             tc.tile_pool(name="orp", bufs=3) as orp, \
             tc.tile_pool(name="tmpp", bufs=3) as tmpp, \
             tc.tile_pool(name="fop", bufs=4) as fop, \
             tc.tile_pool(name="ps_t", bufs=1, space="PSUM") as ps_t, \
             tc.tile_pool(name="ps_dot", bufs=2, space="PSUM") as ps_dot, \
             tc.tile_pool(name="ps_y", bufs=3, space="PSUM") as ps_y, \
             tc.tile_pool(name="ps_f", bufs=2, space="PSUM") as ps_f:

            nc.sync.dma_start(
                out=wp_sb, in_=wprojT.rearrange("(a p) m -> p a m", p=128)
            )
            nc.sync.dma_start(
                out=bias_sb, in_=bproj.rearrange("(a b) o -> b (a o)", b=128)
            )

            # hoisted prep for ALL pairs: bf16 landmarks + odd-head k/v swap
            vones = {}
            for g in range(4):
                for e in range(2):
                    h = 2 * g + e
                    nc.any.tensor_copy(klmb[:, h, :], klm[:, h, :])
                    if e == 1:
                        nc.sync.dma_start(
                            out=klm_fix[64:128, g, :], in_=klmb[0:64, h, :]
                        )
                        nc.sync.dma_start(
                            out=klm_fix[0:64, g, :], in_=klmb[64:128, h, :]
                        )

            def gen_vones(g):
                # vones: [lm(128), 65] = [vlmT | 1] per (head, lm-half).
                # Generator so later pairs' transposes (PE) can interleave
                # into the ACT-paced dot stretch of earlier pairs.
                for e in range(2):
                    for half in range(2):
                        vt = vop.tile(
                            [128, 65], BF16, tag="vones", name=f"vt{g}_{e}_{half}"
                        )
                        pt = ps_t.tile([128, 64], BF16, tag="pt")
                        if e == 0:
                            vsrc = klmb[64:128, 2 * g, bass.ts(half, 128)]
                            idn = ident[64:128, 64:128]
                        else:
                            vsrc = klm_fix[0:64, g, bass.ts(half, 128)]
                            idn = ident[0:64, 0:64]
                        nc.tensor.transpose(pt, vsrc, idn)
                        nc.vector.tensor_copy(vt[:, 0:64], pt)
                        nc.vector.memset(vt[:, 64:65], 1.0)
                        vones[(g, e, half)] = vt
                        yield

            def gen_dot_exp(g, ex):
                # dot + exp, heads interleaved (row-groups 0:64 / 64:128).
                # Generator: yields after each MM+exp so the driver can
                # interleave proj matmuls of an earlier pair (the dot
                # stretch alone is ACT-paced at ~3x the PE time).
                for nt in range(8):
                    for half in range(2):
                        for e in range(2):
                            pd = ps_dot.tile([128, 512], F32, tag="pd")
                            if e == 0:
                                klmh = klmb[0:64, 2 * g, bass.ts(half, 128)]
                                qh = qT[g][0:64, bass.ts(nt, 512)]
                            else:
                                klmh = klm_fix[64:128, g, bass.ts(half, 128)]
                                qh = qT[g][64:128, bass.ts(nt, 512)]
                            nc.tensor.matmul(pd, klmh, qh, start=True, stop=True)
                            nc.scalar.activation(
                                ex[(e, half)][:, bass.ts(nt, 512)],
                                pd,
                                FE.Exp,
                                scale=0.125,
                            )
                            yield

            def gen_y(g, ex, ort):
                # y + rowsum (j-pairs); normalize fully in SBUF.
                # rowsum rows of a group of 2 j-pairs land in one yrh tile
                # (partition 64), get DMA-gathered to partition 0
                # (partition_broadcast only broadcasts absolute partition 0
                # on HW), broadcast on GpSimd, reciprocal in place, then one
                # wide mul per parity. Muls alternate DVE/GpSimd per group
                # to balance the two engines. Yields after each matmul.
                gsz = 2
                for e in range(2):
                    for ihg in range(4):
                        tmph = tmpp.tile(
                            [64, gsz, K_LM], BF16, tag="tmp",
                            name=f"tmp{g}_{e}_{ihg}",
                        )
                        yrh = yrp.tile([65, gsz, 2, K_LM], F32, tag="yrh")
                        for il in range(gsz):
                            i = ihg * gsz + il
                            py = ps_y.tile([65, 2, K_LM], F32, tag="py")
                            for half in range(2):
                                rhs = ex[(e, half)].rearrange(
                                    "p (q j) -> p j q", j=16
                                )[:, 2 * i : 2 * i + 2, :]
                                nc.tensor.matmul(
                                    py,
                                    vones[(g, e, half)],
                                    rhs,
                                    start=(half == 0),
                                    stop=(half == 1),
                                )
                                yield
                            nc.vector.tensor_copy(yrh[:, il, :, :], py)
                        rs0 = rsp.tile([1, gsz, 2, K_LM], F32, tag="rs0")
                        nc.sync.dma_start(out=rs0, in_=yrh[64:65, :, :, :])
                        rc = rcp.tile([64, gsz, 2, K_LM], F32, tag="rc")
                        nc.gpsimd.partition_broadcast(rc, rs0)
                        rc2 = rc.rearrange("p a b q -> p (a b q)")
                        nc.vector.reciprocal_approx_fast(out=rc2, in_=rc2)
                        isl = bass.ds(ihg * gsz, gsz)
                        eng_even = nc.vector if ihg % 2 == 0 else nc.gpsimd
                        eng_odd = nc.gpsimd if ihg % 2 == 0 else nc.vector
                        eng_even.tensor_mul(
                            ort[0:64, isl, e, :],
                            yrh[0:64, :, 0, :],
                            rc[:, :, 0, :],
                        )
                        eng_odd.tensor_mul(
                            tmph[:, :, :],
                            yrh[0:64, :, 1, :],
                            rc[:, :, 1, :],
                        )
                        nc.sync.dma_start(
                            out=ort[64:128, isl, e, :], in_=tmph
                        )

            def gen_proj(g, ort):
                # projection for this head pair (512 output rows);
                # yields after each MM for interleaving
                for co in range(8):
                    pf = ps_f.tile([128, 512], F32, tag="pf")
                    for ci in range(8):
                        nc.tensor.matmul(
                            pf,
                            wp_sb[:, ci, co * 128 : (co + 1) * 128],
                            ort[:, ci, :, :],
                            start=(ci == 0),
                            stop=(ci == 7),
                        )
                        yield
                    fo = fop.tile([128, 512], BF16, tag="fo")
                    nc.scalar.activation(
                        fo, pf, FE.Identity, bias=bias_sb[:, co : co + 1]
                    )
                    nc.sync.dma_start(out=outTt[g, co], in_=fo)

            # 3-deep software pipeline: in driver round g, the in-order PE
            # stream interleaves dot(g) (ACT-paced), y(g-1) (DVE-paced,
            # front-loaded: its ex is complete), and proj(g-2) (back-loaded:
            # its ort finishes during this round).  vones(g+1) transposes
            # ride along.  Steady-state rounds carry ~27us of PE work.
            orts = {}
            exs = {}
            for _ in gen_vones(0):
                pass
            for g in range(6):
                gens = []
                if g < 4:
                    exs[g] = {
                        (e, half): expp.tile(
                            [128, N], BF16, tag="ex", name=f"ex{g}_{e}_{half}"
                        )
                        for e in range(2)
                        for half in range(2)
                    }
                    gens.append((gen_dot_exp(g, exs[g]), 1.0))
                if 0 <= g - 1 < 4:
                    orts[g - 1] = orp.tile(
                        [128, 8, 2, K_LM], BF16, tag="ort", name=f"ort{g - 1}"
                    )
                    gens.append((gen_y(g - 1, exs.pop(g - 1), orts[g - 1]), 1.2))
                if 0 <= g - 2 < 4:
                    gens.append((gen_proj(g - 2, orts.pop(g - 2)), 0.8))
                if g < 3:
                    gens.append((gen_vones(g + 1), 3.0))
                # weighted round-robin: rate = yields per iteration;
                # higher rate drains a generator earlier in the round
                credits = [0.0 for _ in gens]
                done = [False] * len(gens)
                while not all(done):
                    for idx, (gen, rate) in enumerate(gens):
                        if done[idx]:
                            continue
                        credits[idx] += rate
                        while credits[idx] >= 1.0:
                            credits[idx] -= 1.0
                            if next(gen, StopIteration) is StopIteration:
                                done[idx] = True
                                break

    nc.compile()
    return nc


_NC_CACHE = None


def make_in_maps(x, Wqkv, E, Wproj, bproj_v):
    wprojT = np.ascontiguousarray(Wproj.T).astype(ml_dtypes.bfloat16)
    bp = np.ascontiguousarray(bproj_v.reshape(C, 1)).astype(np.float32)

    in_maps = []
    for cid in range(8):
        b, g = cid // 2, cid % 2
        rows = np.concatenate(
            [
                np.arange(s * C + g * HPC * D, s * C + g * HPC * D + HPC * D)
                for s in range(3)
            ]
        )
        # xTt[nb, p, ct, n'] = x[b, nb*512+n', ct*128+p]
        xtt = np.ascontiguousarray(
            x[b].reshape(8, 512, 8, 128).transpose(0, 3, 2, 1)
        ).astype(ml_dtypes.bfloat16)
        # eTt[nb, p, h*4+s, k] = E[g*8+h, k, nb*512+s*128+p]
        eg = E[g * HPC : (g + 1) * HPC]  # [8, 256, 4096]
        ett = np.ascontiguousarray(
            eg.transpose(2, 0, 1)  # [n, h, k]
            .reshape(8, 4, 128, HPC, K_LM)  # [nb, s, p, h, k]
            .transpose(0, 2, 3, 1, 4)  # [nb, p, h, s, k]
            .reshape(8, 128, 32, K_LM)
        ).astype(ml_dtypes.bfloat16)
        in_maps.append(
            {
                "xTt": xtt,
                "wqkvT": np.ascontiguousarray(Wqkv[rows].T).astype(ml_dtypes.bfloat16),
                "eTt": ett,
                "wprojT": wprojT,
                "bproj": bp,
            }
        )
    return in_maps


def kernel(x, Wqkv, E, Wproj, bproj, **_):
    global _NC_CACHE
    x = np.asarray(x, dtype=np.float32)
    Wqkv = np.asarray(Wqkv, dtype=np.float32)
    E = np.asarray(E, dtype=np.float32)
    Wproj = np.asarray(Wproj, dtype=np.float32)
    bproj = np.asarray(bproj, dtype=np.float32)

    if _NC_CACHE is None:
        _NC_CACHE = build_program()
    nc = _NC_CACHE

    in_maps = make_in_maps(x, Wqkv, E, Wproj, bproj)
    res = run_bass_kernel_spmd(nc, in_maps, core_ids=list(range(8)))

    out = np.empty((B, N, C), dtype=np.float32)
    for cid in range(8):
        b, g = cid // 2, cid % 2
        arr = np.asarray(res.results[cid]["outTt"], dtype=np.float32)
        # [gp, co, p, e, q] -> rows (2gp+e)*256+q, cols co*128+p
        out[b, g * 2048 : (g + 1) * 2048, :] = (
            arr.reshape(4, 8, 128, 2, 256)
            .transpose(0, 3, 4, 1, 2)
            .reshape(2048, 1024)
        )
    return out
